# revision 9
# baseline (speedup 1.0000x reference)
"""Trainium2 Bass kernel for nn_AttentionSeqToMasked (dense transformer attention).

Full-input contract: kernel(**inputs) takes the unsharded numpy inputs and
returns the full [B, SQ, H*D_V] float32 output.

Sharding (8 cores): data parallel over batch (B=4 -> 2 cores per batch) x
tensor parallel over heads (16 heads -> 8 per core). Each core computes
attention for one (batch, head-half) pair; host gathers the slices.

Per-core dataflow (all matmuls bf16 inputs, fp32 PSUM accumulation):
  - Host pre-transposes activations to X^T [D_PRE, S] bf16 so the contraction
    dim (D_PRE) lands on SBUF partitions with fully-contiguous DMA loads.
  - Projections compute q^T/k^T = W^T @ X^T directly (head-dim on partitions),
    v in natural [s, d_v] layout with a ones-column appended via the weight
    matrix (zero weight column + bias 1.0).
  - Scores are computed transposed: scoresT[k, q] = kT.T @ qT, two heads
    packed into the 128x128 PE array per matmul pair (d_head=64 row groups).
  - Key-mask folds into the exp as a per-partition bias (0 or -30000);
    1/sqrt(d) folds into the exp scale. No max-subtraction is needed
    (logits are O(1) by construction; exp cannot overflow fp32).
  - AV matmul contracts exp(scores)T with [v | ones]: row 64 of the psum is
    the softmax denominator, computed for free alongside the numerator.
  - A final PE transpose returns [q, d_v+1] tiles; VectorE multiplies by the
    reciprocal denominator and the result DMAs straight to DRAM.

Scheduling: projection work for pair p+1 is chopped into ~1.7us psum-chunks
and interleaved into pair p's attention loop every 4 key-tiles, keeping the
TensorE fed while ScalarE (exp) is the steady-state bottleneck.
"""

import os
from contextlib import ExitStack

import numpy as np
import ml_dtypes

import concourse.bass as bass
import concourse.bacc as bacc
import concourse.mybir as mybir
import concourse.tile as tile
from concourse.bass_utils import run_bass_kernel_spmd
from concourse.masks import make_identity

# Problem shape (hardcoded per contract)
B, SQ, SK = 4, 2048, 2048
D_PRE = 1024
H, D_QK, D_V = 16, 64, 64
N_CORES = 8
HALF = (H // 2) * D_QK  # 512 columns of the projection handled per core
N_PAIRS = 4  # head pairs per core
S_CHUNK = 512  # moving free-dim per matmul
N_DT = D_PRE // 128  # d_pre tiles of 128
N_KT = SK // 128  # key tiles of 128
N_QC = SQ // S_CHUNK  # query chunks of 512
MASK_NEG = -30000.0

F32 = mybir.dt.float32
BF16 = mybir.dt.bfloat16
BF16_NP = np.dtype(ml_dtypes.bfloat16)

_COMPILED = None


def _build_program():
    nc = bacc.Bacc("TRN2", target_bir_lowering=False, debug=False)

    # DRAM I/O (names are the in_map keys)
    xq = nc.dram_tensor("xq", [D_PRE, SQ], BF16, kind="ExternalInput").ap()
    xk = nc.dram_tensor("xk", [D_PRE, SK], BF16, kind="ExternalInput").ap()
    xv = nc.dram_tensor("xv", [D_PRE, SK], BF16, kind="ExternalInput").ap()
    wq = nc.dram_tensor("wq", [D_PRE, HALF], BF16, kind="ExternalInput").ap()
    wk = nc.dram_tensor("wk", [D_PRE, HALF], BF16, kind="ExternalInput").ap()
    # v weights with a zero column appended per head (ones column generator)
    wv = nc.dram_tensor("wv", [D_PRE, N_PAIRS * 130], BF16, kind="ExternalInput").ap()
    bq = nc.dram_tensor("bq", [128, N_PAIRS], F32, kind="ExternalInput").ap()
    bk = nc.dram_tensor("bk", [128, N_PAIRS], F32, kind="ExternalInput").ap()
    bv = nc.dram_tensor("bv", [128, N_PAIRS * 130], F32, kind="ExternalInput").ap()
    mb = nc.dram_tensor("mb", [128, N_KT], F32, kind="ExternalInput").ap()
    out = nc.dram_tensor("out", [SQ, HALF], F32, kind="ExternalOutput").ap()

    with tile.TileContext(nc) as tc:
        _emit(tc, xq, xk, xv, wq, wk, wv, bq, bk, bv, mb, out)

    nc.compile()
    return nc


def _emit(tc, xq, xk, xv, wq, wk, wv, bq, bk, bv, mb, out):
    nc = tc.nc

    with ExitStack() as ctx:
        # ---- pools ----
        xp = ctx.enter_context(tc.tile_pool(name="x", bufs=3 * N_DT))
        wp = ctx.enter_context(tc.tile_pool(name="w", bufs=1))
        cp = ctx.enter_context(tc.tile_pool(name="const", bufs=1))
        qkvp = ctx.enter_context(tc.tile_pool(name="qkv", bufs=1))
        expp = ctx.enter_context(tc.tile_pool(name="exp", bufs=3))
        avtp = ctx.enter_context(tc.tile_pool(name="avt", bufs=2))
        stgp = ctx.enter_context(tc.tile_pool(name="stg", bufs=4))
        rp = ctx.enter_context(tc.tile_pool(name="recip", bufs=4))

        proj_ps = ctx.enter_context(tc.tile_pool(name="proj_ps", bufs=1, space="PSUM"))
        sc_ps = ctx.enter_context(tc.tile_pool(name="sc_ps", bufs=2, space="PSUM"))
        av_ps = ctx.enter_context(tc.tile_pool(name="av_ps", bufs=2, space="PSUM"))
        tp_ps = ctx.enter_context(tc.tile_pool(name="tp_ps", bufs=1, space="PSUM"))

        # ---- constants ----
        ident = cp.tile([128, 128], F32, name="ident")
        make_identity(nc, ident)
        mb_sb = cp.tile([128, N_KT], F32, name="mb_sb")
        nc.sync.dma_start(mb_sb, mb)
        bq_sb = cp.tile([128, N_PAIRS], F32, name="bq_sb")
        nc.sync.dma_start(bq_sb, bq)
        bk_sb = cp.tile([128, N_PAIRS], F32, name="bk_sb")
        nc.sync.dma_start(bk_sb, bk)
        bv_sb = cp.tile([128, N_PAIRS * 130], F32, name="bv_sb")
        nc.sync.dma_start(bv_sb, bv)

        # ---- streamed loads, ordered so the q projection can start after
        # only wq+xq have landed ----
        def load_x(xap, pfx):
            ts = []
            for dt_i in range(N_DT):
                t = xp.tile([128, SQ], BF16, name=f"{pfx}{dt_i}", tag="x")
                nc.sync.dma_start(t, xap[dt_i * 128 : (dt_i + 1) * 128, :])
                ts.append(t)
            return ts

        def load_w(wap, pfx, width):
            ts = []
            for dt_i in range(N_DT):
                t = wp.tile([128, width], BF16, name=f"{pfx}{dt_i}", tag=f"{pfx}{dt_i}")
                nc.sync.dma_start(t, wap[dt_i * 128 : (dt_i + 1) * 128, :])
                ts.append(t)
            return ts

        wq_sb = load_w(wq, "wq", HALF)
        xq_sb = load_x(xq, "xq")
        wk_sb = load_w(wk, "wk", HALF)
        xk_sb = load_x(xk, "xk")
        wv_sb = load_w(wv, "wv", N_PAIRS * 130)
        xv_sb = load_x(xv, "xv")

        v_tiles = {}  # (pair, kt) -> [128, 130] bf16 tile
        qkT = {}  # (pfx, pair) -> [128, SQ] bf16 tile

        def qk_tile(pfx, pair):
            if (pfx, pair) not in qkT:
                qkT[(pfx, pair)] = qkvp.tile(
                    [128, SQ], BF16, name=f"{pfx}T{pair}", tag=f"{pfx}T", bufs=3
                )
            return qkT[(pfx, pair)]

        def emit_qk_chunk(pair, pfx, qc):
            # one [128, 512] projection chunk: 8 accumulating MMs + bias copy
            dst = qk_tile(pfx, pair)
            w_sb = wq_sb if pfx == "q" else wk_sb
            b_sb = bq_sb if pfx == "q" else bk_sb
            x_sb = xq_sb if pfx == "q" else xk_sb
            ps = proj_ps.tile([128, S_CHUNK], F32, name=f"{pfx}ps{pair}_{qc}", tag="proj")
            for dt_i in range(N_DT):
                nc.tensor.matmul(
                    ps,
                    lhsT=w_sb[dt_i][:, pair * 128 : (pair + 1) * 128],
                    rhs=x_sb[dt_i][:, qc * S_CHUNK : (qc + 1) * S_CHUNK],
                    start=(dt_i == 0),
                    stop=(dt_i == N_DT - 1),
                )
            nc.vector.tensor_scalar_add(
                dst[:, qc * S_CHUNK : (qc + 1) * S_CHUNK],
                ps,
                b_sb[:, pair : pair + 1],
            )

        def emit_v_chunk(g, st):
            # v projection for pairs (2g, 2g+1), one key tile: N=260 matmuls
            ps = proj_ps.tile([128, S_CHUNK], F32, name=f"vps{g}_{st}", tag="proj")
            for dt_i in range(N_DT):
                nc.tensor.matmul(
                    ps[:, 0:260],
                    lhsT=xv_sb[dt_i][:, st * 128 : (st + 1) * 128],
                    rhs=wv_sb[dt_i][:, g * 260 : (g + 1) * 260],
                    start=(dt_i == 0),
                    stop=(dt_i == N_DT - 1),
                )
            for j in range(2):
                pair = 2 * g + j
                vt = qkvp.tile(
                    [128, 130], BF16, name=f"v{pair}_{st}", tag="v", bufs=4 * N_KT
                )
                nc.vector.tensor_add(
                    vt,
                    ps[:, j * 130 : (j + 1) * 130],
                    bv_sb[:, pair * 130 : (pair + 1) * 130],
                )
                v_tiles[(pair, st)] = vt

        # filler queue: projection chunks for pairs >= 1, popped as TensorE
        # filler inside the attention loops (4 slots per query chunk)
        filler = []
        filler += [(lambda p=1, c=c: emit_qk_chunk(p, "q", c)) for c in range(N_QC)]
        filler += [(lambda p=1, c=c: emit_qk_chunk(p, "k", c)) for c in range(N_QC)]
        filler += [(lambda st=st: emit_v_chunk(1, st)) for st in range(N_KT)]
        filler += [(lambda p=2, c=c: emit_qk_chunk(p, "q", c)) for c in range(N_QC)]
        filler += [(lambda p=2, c=c: emit_qk_chunk(p, "k", c)) for c in range(N_QC)]
        filler += [(lambda p=3, c=c: emit_qk_chunk(p, "q", c)) for c in range(N_QC)]
        filler += [(lambda p=3, c=c: emit_qk_chunk(p, "k", c)) for c in range(N_QC)]

        def pop_filler():
            if filler:
                filler.pop(0)()

        # prologue: full projection for pair 0 (+ pair 1's v, same chunks)
        for c in range(N_QC):
            emit_qk_chunk(0, "q", c)
        for c in range(N_QC):
            emit_qk_chunk(0, "k", c)
        for st in range(N_KT):
            emit_v_chunk(0, st)

        for pair in range(N_PAIRS):
            qT = qk_tile("q", pair)
            kT = qk_tile("k", pair)

            # ---- attention for this pair ----
            for qc in range(N_QC):
                av_a = av_ps.tile([65, S_CHUNK], F32, name=f"ava{pair}_{qc}", tag="av")
                av_b = av_ps.tile([65, S_CHUNK], F32, name=f"avb{pair}_{qc}", tag="av")
                for kt2 in range(0, N_KT, 2):
                    kts = (kt2, kt2 + 1)
                    scs = {}
                    for kt in kts:
                        sc = sc_ps.tile(
                            [128, 1024], F32, name=f"sc{pair}_{qc}_{kt}", tag="sc"
                        )
                        scs[kt] = sc
                        # scoresT for heads A and B, packed in PE row groups
                        nc.tensor.matmul(
                            sc[:, 0:512],
                            lhsT=kT[0:64, kt * 128 : (kt + 1) * 128],
                            rhs=qT[0:64, qc * S_CHUNK : (qc + 1) * S_CHUNK],
                            start=True,
                            stop=True,
                        )
                        nc.tensor.matmul(
                            sc[:, 512:1024],
                            lhsT=kT[64:128, kt * 128 : (kt + 1) * 128],
                            rhs=qT[64:128, qc * S_CHUNK : (qc + 1) * S_CHUNK],
                            start=True,
                            stop=True,
                        )
                    exs = {}
                    for kt in kts:
                        ex = expp.tile(
                            [128, 1024], BF16, name=f"ex{pair}_{qc}_{kt}", tag="ex"
                        )
                        exs[kt] = ex
                        nc.scalar.activation(
                            ex,
                            scs[kt],
                            mybir.ActivationFunctionType.Exp,
                            bias=mb_sb[:, kt : kt + 1],
                            scale=0.125,
                        )
                    for kt in kts:
                        nc.tensor.matmul(
                            av_a,
                            lhsT=v_tiles[(pair, kt)][:, 0:65],
                            rhs=exs[kt][:, 0:512],
                            start=(kt == 0),
                            stop=(kt == N_KT - 1),
                        )
                        nc.tensor.matmul(
                            av_b,
                            lhsT=v_tiles[(pair, kt)][:, 65:130],
                            rhs=exs[kt][:, 512:1024],
                            start=(kt == 0),
                            stop=(kt == N_KT - 1),
                        )
                    if kt2 % 4 == 2:
                        pop_filler()

                # transpose back to [q, d_v], normalize, store
                stgs = [
                    stgp.tile([128, 128], F32, name=f"st{pair}_{qc}_{u}", tag="stg")
                    for u in range(4)
                ]
                for h_i, av in enumerate((av_a, av_b)):
                    avt = avtp.tile(
                        [65, S_CHUNK], F32, name=f"avt{pair}_{qc}_{h_i}", tag="avt"
                    )
                    nc.vector.tensor_copy(avt, av)
                    tp = tp_ps.tile([128, 260], F32, name=f"tp{pair}_{qc}_{h_i}", tag="tp")
                    for u in range(4):
                        nc.tensor.transpose(
                            tp[:, u * 65 : u * 65 + 65],
                            avt[:, u * 128 : (u + 1) * 128],
                            ident[0:65, 0:65],
                        )
                    for u in range(4):
                        rc = rp.tile([128, 1], F32, name=f"rc{pair}_{qc}_{h_i}_{u}", tag="rc")
                        nc.vector.reciprocal(rc, tp[:, u * 65 + 64 : u * 65 + 65])
                        nc.vector.tensor_scalar_mul(
                            stgs[u][:, h_i * 64 : (h_i + 1) * 64],
                            tp[:, u * 65 : u * 65 + 64],
                            rc,
                        )
                for u in range(4):
                    qt = qc * 4 + u
                    nc.sync.dma_start(
                        out[qt * 128 : (qt + 1) * 128, pair * 128 : (pair + 1) * 128],
                        stgs[u],
                    )

        assert not filler, f"{len(filler)} filler chunks left unscheduled"


def _prep_core_inputs(pre_qs, pre_ks, pre_vs, k_mask, q_w, q_b, k_w, k_b, v_w, v_b, core):
    b = core // 2
    hh = core % 2
    cols = slice(HALF * hh, HALF * (hh + 1))

    xq = np.ascontiguousarray(pre_qs[b].T).astype(BF16_NP)
    xk = np.ascontiguousarray(pre_ks[b].T).astype(BF16_NP)
    xv = np.ascontiguousarray(pre_vs[b].T).astype(BF16_NP)
    wq = np.ascontiguousarray(q_w[:, cols]).astype(BF16_NP)
    wk = np.ascontiguousarray(k_w[:, cols]).astype(BF16_NP)

    wv_core = v_w[:, cols].astype(np.float32)
    wv = np.zeros((D_PRE, N_PAIRS * 130), dtype=np.float32)
    bv_core = v_b[cols].astype(np.float32)
    bv_ext = np.zeros(N_PAIRS * 130, dtype=np.float32)
    for p in range(N_PAIRS):
        wv[:, p * 130 : p * 130 + 64] = wv_core[:, p * 128 : p * 128 + 64]
        wv[:, p * 130 + 65 : p * 130 + 129] = wv_core[:, p * 128 + 64 : p * 128 + 128]
        bv_ext[p * 130 : p * 130 + 64] = bv_core[p * 128 : p * 128 + 64]
        bv_ext[p * 130 + 64] = 1.0
        bv_ext[p * 130 + 65 : p * 130 + 129] = bv_core[p * 128 + 64 : p * 128 + 128]
        bv_ext[p * 130 + 129] = 1.0

    bq = np.ascontiguousarray(q_b[cols].astype(np.float32).reshape(N_PAIRS, 128).T)
    bk = np.ascontiguousarray(k_b[cols].astype(np.float32).reshape(N_PAIRS, 128).T)
    bv_full = np.ascontiguousarray(np.tile(bv_ext[None, :], (128, 1)))

    # mask True -> 0.0, False -> MASK_NEG
    mbias = np.where(k_mask[b], 0.0, MASK_NEG).astype(np.float32)
    mb = np.ascontiguousarray(mbias.reshape(N_KT, 128).T)

    return {
        "xq": xq,
        "xk": xk,
        "xv": xv,
        "wq": wq,
        "wk": wk,
        "wv": wv.astype(BF16_NP),
        "bq": bq,
        "bk": bk,
        "bv": bv_full,
        "mb": mb,
    }


def kernel(pre_qs, pre_ks, pre_vs, k_mask, q_w, q_b, k_w, k_b, v_w, v_b):
    global _COMPILED
    args = (pre_qs, pre_ks, pre_vs, k_mask, q_w, q_b, k_w, k_b, v_w, v_b)
    args = tuple(np.asarray(a) for a in args)

    if _COMPILED is None:
        _COMPILED = _build_program()
    nc = _COMPILED

    in_maps = [_prep_core_inputs(*args, core=c) for c in range(N_CORES)]

    trace = bool(int(os.environ.get("BASS_KERNEL_TRACE", "0")))
    res = run_bass_kernel_spmd(
        nc,
        in_maps,
        core_ids=list(range(N_CORES)),
        trace=trace,
    )
    if trace:
        kernel.last_results = res

    out = np.empty((B, SQ, H * D_V), dtype=np.float32)
    for c in range(N_CORES):
        b = c // 2
        hh = c % 2
        out[b, :, HALF * hh : HALF * (hh + 1)] = res.results[c]["out"]
    return out


# revision 12
# speedup vs baseline: 1.0211x; 1.0211x over previous
"""Trainium2 Bass kernel for nn_AttentionSeqToMasked (dense transformer attention).

Full-input contract: kernel(**inputs) takes the unsharded numpy inputs and
returns the full [B, SQ, H*D_V] float32 output.

Sharding (8 cores): data parallel over batch (B=4 -> 2 cores per batch) x
tensor parallel over heads (16 heads -> 8 per core). Each core computes
attention for one (batch, head-half) pair; host gathers the slices.

Per-core dataflow (all matmuls bf16 inputs, fp32 PSUM accumulation):
  - Host pre-transposes activations to X^T [D_PRE, S] bf16 so the contraction
    dim (D_PRE) lands on SBUF partitions with fully-contiguous DMA loads.
  - Projections compute q^T/k^T = W^T @ X^T directly (head-dim on partitions),
    v in natural [s, d_v] layout with a ones-column appended via the weight
    matrix (zero weight column + bias 1.0).
  - Scores are computed transposed: scoresT[k, q] = kT.T @ qT, two heads
    packed into the 128x128 PE array per matmul pair (d_head=64 row groups).
  - Key-mask folds into the exp as a per-partition bias (0 or -30000);
    1/sqrt(d) folds into the exp scale. No max-subtraction is needed
    (logits are O(1) by construction; exp cannot overflow fp32).
  - AV matmul contracts exp(scores)T with [v | ones]: row 64 of the psum is
    the softmax denominator, computed for free alongside the numerator.
  - A final PE transpose returns [q, d_v+1] tiles; VectorE multiplies by the
    reciprocal denominator and the result DMAs straight to DRAM.

Scheduling: projection work for pair p+1 is chopped into ~1.7us psum-chunks
and interleaved into pair p's attention loop every 4 key-tiles, keeping the
TensorE fed while ScalarE (exp) is the steady-state bottleneck.
"""

import os
from contextlib import ExitStack

import numpy as np
import ml_dtypes

import concourse.bass as bass
import concourse.bacc as bacc
import concourse.mybir as mybir
import concourse.tile as tile
from concourse.bass_utils import run_bass_kernel_spmd
from concourse.masks import make_identity

# Problem shape (hardcoded per contract)
B, SQ, SK = 4, 2048, 2048
D_PRE = 1024
H, D_QK, D_V = 16, 64, 64
N_CORES = 8
HALF = (H // 2) * D_QK  # 512 columns of the projection handled per core
N_PAIRS = 4  # head pairs per core
S_CHUNK = 512  # moving free-dim per matmul
N_DT = D_PRE // 128  # d_pre tiles of 128
N_KT = SK // 128  # key tiles of 128
N_QC = SQ // S_CHUNK  # query chunks of 512
MASK_NEG = -30000.0

F32 = mybir.dt.float32
BF16 = mybir.dt.bfloat16
BF16_NP = np.dtype(ml_dtypes.bfloat16)

_COMPILED = None


def _build_program():
    nc = bacc.Bacc("TRN2", target_bir_lowering=False, debug=False)

    # DRAM I/O (names are the in_map keys)
    xq = nc.dram_tensor("xq", [D_PRE, SQ], BF16, kind="ExternalInput").ap()
    xk = nc.dram_tensor("xk", [D_PRE, SK], BF16, kind="ExternalInput").ap()
    xv = nc.dram_tensor("xv", [D_PRE, SK], BF16, kind="ExternalInput").ap()
    wq = nc.dram_tensor("wq", [D_PRE, HALF], BF16, kind="ExternalInput").ap()
    wk = nc.dram_tensor("wk", [D_PRE, HALF], BF16, kind="ExternalInput").ap()
    # v weights with a zero column appended per head (ones column generator)
    wv = nc.dram_tensor("wv", [D_PRE, N_PAIRS * 130], BF16, kind="ExternalInput").ap()
    bq = nc.dram_tensor("bq", [128, N_PAIRS], F32, kind="ExternalInput").ap()
    bk = nc.dram_tensor("bk", [128, N_PAIRS], F32, kind="ExternalInput").ap()
    bv = nc.dram_tensor("bv", [128, N_PAIRS * 130], F32, kind="ExternalInput").ap()
    mb = nc.dram_tensor("mb", [128, N_KT], F32, kind="ExternalInput").ap()
    out = nc.dram_tensor("out", [SQ, HALF], F32, kind="ExternalOutput").ap()

    with tile.TileContext(nc) as tc:
        _emit(tc, xq, xk, xv, wq, wk, wv, bq, bk, bv, mb, out)

    nc.compile()
    return nc


def _emit(tc, xq, xk, xv, wq, wk, wv, bq, bk, bv, mb, out):
    nc = tc.nc

    with ExitStack() as ctx:
        # ---- pools ----
        xp = ctx.enter_context(tc.tile_pool(name="x", bufs=3 * N_DT))
        wp = ctx.enter_context(tc.tile_pool(name="w", bufs=1))
        cp = ctx.enter_context(tc.tile_pool(name="const", bufs=1))
        qkvp = ctx.enter_context(tc.tile_pool(name="qkv", bufs=1))
        expp = ctx.enter_context(tc.tile_pool(name="exp", bufs=3))
        avtp = ctx.enter_context(tc.tile_pool(name="avt", bufs=2))
        stgp = ctx.enter_context(tc.tile_pool(name="stg", bufs=8))
        rp = ctx.enter_context(tc.tile_pool(name="recip", bufs=8))

        proj_ps = ctx.enter_context(tc.tile_pool(name="proj_ps", bufs=1, space="PSUM"))
        sc_ps = ctx.enter_context(tc.tile_pool(name="sc_ps", bufs=2, space="PSUM"))
        av_ps = ctx.enter_context(tc.tile_pool(name="av_ps", bufs=2, space="PSUM"))
        tp_ps = ctx.enter_context(tc.tile_pool(name="tp_ps", bufs=1, space="PSUM"))

        # ---- constants ----
        ident = cp.tile([128, 128], F32, name="ident")
        make_identity(nc, ident)
        mb_sb = cp.tile([128, N_KT], F32, name="mb_sb")
        nc.sync.dma_start(mb_sb, mb)
        bq_sb = cp.tile([128, N_PAIRS], F32, name="bq_sb")
        nc.sync.dma_start(bq_sb, bq)
        bk_sb = cp.tile([128, N_PAIRS], F32, name="bk_sb")
        nc.sync.dma_start(bk_sb, bk)
        bv_sb = cp.tile([128, N_PAIRS * 130], F32, name="bv_sb")
        nc.sync.dma_start(bv_sb, bv)

        # ---- streamed loads, ordered so the q projection can start after
        # only wq+xq have landed ----
        def load_x(xap, pfx):
            ts = []
            for dt_i in range(N_DT):
                t = xp.tile([128, SQ], BF16, name=f"{pfx}{dt_i}", tag="x")
                nc.sync.dma_start(t, xap[dt_i * 128 : (dt_i + 1) * 128, :])
                ts.append(t)
            return ts

        def load_w(wap, pfx, width):
            ts = []
            for dt_i in range(N_DT):
                t = wp.tile([128, width], BF16, name=f"{pfx}{dt_i}", tag=f"{pfx}{dt_i}")
                nc.sync.dma_start(t, wap[dt_i * 128 : (dt_i + 1) * 128, :])
                ts.append(t)
            return ts

        wq_sb = load_w(wq, "wq", HALF)
        xq_sb = load_x(xq, "xq")
        wk_sb = load_w(wk, "wk", HALF)
        xk_sb = load_x(xk, "xk")
        wv_sb = load_w(wv, "wv", N_PAIRS * 130)
        xv_sb = load_x(xv, "xv")

        v_tiles = {}  # (pair, kt) -> [128, 130] bf16 tile
        qkT = {}  # (pfx, pair) -> [128, SQ] bf16 tile

        def qk_tile(pfx, pair):
            if (pfx, pair) not in qkT:
                qkT[(pfx, pair)] = qkvp.tile(
                    [128, SQ], BF16, name=f"{pfx}T{pair}", tag=f"{pfx}T", bufs=3
                )
            return qkT[(pfx, pair)]

        proj_ps_open = {}

        def emit_qk_chunk(pair, pfx, qc, half=None):
            # one [128, 512] projection chunk: 8 accumulating MMs + bias copy.
            # half=0/1 emits only the first/second 4 contraction MMs (filler
            # granularity); half=None emits the whole chunk.
            dst = qk_tile(pfx, pair)
            w_sb = wq_sb if pfx == "q" else wk_sb
            b_sb = bq_sb if pfx == "q" else bk_sb
            x_sb = xq_sb if pfx == "q" else xk_sb
            key = (pair, pfx, qc)
            if half == 1:
                ps = proj_ps_open.pop(key)
            else:
                ps = proj_ps.tile(
                    [128, S_CHUNK], F32, name=f"{pfx}ps{pair}_{qc}", tag="proj"
                )
            dts = range(N_DT) if half is None else range(half * 4, half * 4 + 4)
            for dt_i in dts:
                nc.tensor.matmul(
                    ps,
                    lhsT=w_sb[dt_i][:, pair * 128 : (pair + 1) * 128],
                    rhs=x_sb[dt_i][:, qc * S_CHUNK : (qc + 1) * S_CHUNK],
                    start=(dt_i == 0),
                    stop=(dt_i == N_DT - 1),
                )
            if half == 0:
                proj_ps_open[key] = ps
            else:
                nc.vector.tensor_scalar_add(
                    dst[:, qc * S_CHUNK : (qc + 1) * S_CHUNK],
                    ps,
                    b_sb[:, pair : pair + 1],
                )

        def emit_v_chunk(g, st):
            # v projection for pairs (2g, 2g+1), one key tile: N=260 matmuls
            ps = proj_ps.tile([128, S_CHUNK], F32, name=f"vps{g}_{st}", tag="proj")
            for dt_i in range(N_DT):
                nc.tensor.matmul(
                    ps[:, 0:260],
                    lhsT=xv_sb[dt_i][:, st * 128 : (st + 1) * 128],
                    rhs=wv_sb[dt_i][:, g * 260 : (g + 1) * 260],
                    start=(dt_i == 0),
                    stop=(dt_i == N_DT - 1),
                )
            for j in range(2):
                pair = 2 * g + j
                vt = qkvp.tile(
                    [128, 130], BF16, name=f"v{pair}_{st}", tag="v", bufs=4 * N_KT
                )
                nc.vector.tensor_add(
                    vt,
                    ps[:, j * 130 : (j + 1) * 130],
                    bv_sb[:, pair * 130 : (pair + 1) * 130],
                )
                v_tiles[(pair, st)] = vt

        # filler queue: projection chunk units (~0.85us of PE each), popped as
        # TensorE filler inside pair-0/1 attention (every other key tile).
        # qk chunks are split into 4-MM halves to match the per-slot budget.
        filler = []

        def _qk_half(pair, pfx, qc, half):
            return lambda: emit_qk_chunk(pair, pfx, qc, half)

        for p in (1, 2):
            for pfx in ("q", "k"):
                for c in range(N_QC):
                    filler.append(_qk_half(p, pfx, c, 0))
                    filler.append(_qk_half(p, pfx, c, 1))
        filler += [(lambda st=st: emit_v_chunk(1, st)) for st in range(N_KT)]
        for pfx in ("q", "k"):
            for c in range(N_QC):
                filler.append(_qk_half(3, pfx, c, 0))
                filler.append(_qk_half(3, pfx, c, 1))

        def pop_filler():
            if filler:
                filler.pop(0)()

        # prologue: full projection for pair 0 (+ pair 1's v, same chunks)
        for c in range(N_QC):
            emit_qk_chunk(0, "q", c)
        for c in range(N_QC):
            emit_qk_chunk(0, "k", c)
        for st in range(N_KT):
            emit_v_chunk(0, st)

        # ---- software-pipelined attention stream over (pair, qc, kt) ----
        iters = [
            (pair, qc, kt)
            for pair in range(N_PAIRS)
            for qc in range(N_QC)
            for kt in range(N_KT)
        ]
        sc_map = {}
        av_map = {}

        def emit_scores(i):
            pair, qc, kt = iters[i]
            qT = qk_tile("q", pair)
            kT = qk_tile("k", pair)
            sc = sc_ps.tile([128, 1024], F32, name=f"sc{pair}_{qc}_{kt}", tag="sc")
            # scoresT for heads A and B, packed in PE row groups
            nc.tensor.matmul(
                sc[:, 0:512],
                lhsT=kT[0:64, kt * 128 : (kt + 1) * 128],
                rhs=qT[0:64, qc * S_CHUNK : (qc + 1) * S_CHUNK],
                start=True,
                stop=True,
            )
            nc.tensor.matmul(
                sc[:, 512:1024],
                lhsT=kT[64:128, kt * 128 : (kt + 1) * 128],
                rhs=qT[64:128, qc * S_CHUNK : (qc + 1) * S_CHUNK],
                start=True,
                stop=True,
            )
            sc_map[i] = sc

        def emit_epilogue(pair, qc, av_a, av_b):
            # transpose back to [q, d_v], normalize, store
            stgs = [
                stgp.tile([128, 128], F32, name=f"st{pair}_{qc}_{u}", tag="stg")
                for u in range(4)
            ]
            for h_i, av in enumerate((av_a, av_b)):
                avt = avtp.tile(
                    [65, S_CHUNK], F32, name=f"avt{pair}_{qc}_{h_i}", tag="avt"
                )
                nc.vector.tensor_copy(avt, av)
                tp = tp_ps.tile([128, 260], F32, name=f"tp{pair}_{qc}_{h_i}", tag="tp")
                for u in range(4):
                    nc.tensor.transpose(
                        tp[:, u * 65 : u * 65 + 65],
                        avt[:, u * 128 : (u + 1) * 128],
                        ident[0:65, 0:65],
                    )
                for u in range(4):
                    rc = rp.tile([128, 1], F32, name=f"rc{pair}_{qc}_{h_i}_{u}", tag="rc")
                    nc.vector.reciprocal(rc, tp[:, u * 65 + 64 : u * 65 + 65])
                    nc.vector.tensor_scalar_mul(
                        stgs[u][:, h_i * 64 : (h_i + 1) * 64],
                        tp[:, u * 65 : u * 65 + 64],
                        rc,
                    )
            for u in range(4):
                qt = qc * 4 + u
                nc.sync.dma_start(
                    out[qt * 128 : (qt + 1) * 128, pair * 128 : (pair + 1) * 128],
                    stgs[u],
                )

        emit_scores(0)
        emit_scores(1)
        for i, (pair, qc, kt) in enumerate(iters):
            ex = expp.tile([128, 1024], BF16, name=f"ex{pair}_{qc}_{kt}", tag="ex", bufs=6)
            nc.scalar.activation(
                ex,
                sc_map.pop(i),
                mybir.ActivationFunctionType.Exp,
                bias=mb_sb[:, kt : kt + 1],
                scale=0.125,
            )
            if kt == 0:
                av_map[(pair, qc)] = (
                    av_ps.tile([65, S_CHUNK], F32, name=f"ava{pair}_{qc}", tag="av"),
                    av_ps.tile([65, S_CHUNK], F32, name=f"avb{pair}_{qc}", tag="av"),
                )
            av_a, av_b = av_map[(pair, qc)]
            nc.tensor.matmul(
                av_a,
                lhsT=v_tiles[(pair, kt)][:, 0:65],
                rhs=ex[:, 0:512],
                start=(kt == 0),
                stop=(kt == N_KT - 1),
            )
            nc.tensor.matmul(
                av_b,
                lhsT=v_tiles[(pair, kt)][:, 65:130],
                rhs=ex[:, 512:1024],
                start=(kt == 0),
                stop=(kt == N_KT - 1),
            )
            if i + 2 < len(iters):
                emit_scores(i + 2)
            if kt % 2 == 1 and pair <= 1:
                pop_filler()
            if kt == N_KT - 1:
                emit_epilogue(pair, qc, *av_map.pop((pair, qc)))

        assert not filler, f"{len(filler)} filler chunks left unscheduled"


def _prep_core_inputs(pre_qs, pre_ks, pre_vs, k_mask, q_w, q_b, k_w, k_b, v_w, v_b, core):
    b = core // 2
    hh = core % 2
    cols = slice(HALF * hh, HALF * (hh + 1))

    xq = np.ascontiguousarray(pre_qs[b].T).astype(BF16_NP)
    xk = np.ascontiguousarray(pre_ks[b].T).astype(BF16_NP)
    xv = np.ascontiguousarray(pre_vs[b].T).astype(BF16_NP)
    wq = np.ascontiguousarray(q_w[:, cols]).astype(BF16_NP)
    wk = np.ascontiguousarray(k_w[:, cols]).astype(BF16_NP)

    wv_core = v_w[:, cols].astype(np.float32)
    wv = np.zeros((D_PRE, N_PAIRS * 130), dtype=np.float32)
    bv_core = v_b[cols].astype(np.float32)
    bv_ext = np.zeros(N_PAIRS * 130, dtype=np.float32)
    for p in range(N_PAIRS):
        wv[:, p * 130 : p * 130 + 64] = wv_core[:, p * 128 : p * 128 + 64]
        wv[:, p * 130 + 65 : p * 130 + 129] = wv_core[:, p * 128 + 64 : p * 128 + 128]
        bv_ext[p * 130 : p * 130 + 64] = bv_core[p * 128 : p * 128 + 64]
        bv_ext[p * 130 + 64] = 1.0
        bv_ext[p * 130 + 65 : p * 130 + 129] = bv_core[p * 128 + 64 : p * 128 + 128]
        bv_ext[p * 130 + 129] = 1.0

    bq = np.ascontiguousarray(q_b[cols].astype(np.float32).reshape(N_PAIRS, 128).T)
    bk = np.ascontiguousarray(k_b[cols].astype(np.float32).reshape(N_PAIRS, 128).T)
    bv_full = np.ascontiguousarray(np.tile(bv_ext[None, :], (128, 1)))

    # mask True -> 0.0, False -> MASK_NEG
    mbias = np.where(k_mask[b], 0.0, MASK_NEG).astype(np.float32)
    mb = np.ascontiguousarray(mbias.reshape(N_KT, 128).T)

    return {
        "xq": xq,
        "xk": xk,
        "xv": xv,
        "wq": wq,
        "wk": wk,
        "wv": wv.astype(BF16_NP),
        "bq": bq,
        "bk": bk,
        "bv": bv_full,
        "mb": mb,
    }


def kernel(pre_qs, pre_ks, pre_vs, k_mask, q_w, q_b, k_w, k_b, v_w, v_b):
    global _COMPILED
    args = (pre_qs, pre_ks, pre_vs, k_mask, q_w, q_b, k_w, k_b, v_w, v_b)
    args = tuple(np.asarray(a) for a in args)

    if _COMPILED is None:
        _COMPILED = _build_program()
    nc = _COMPILED

    in_maps = [_prep_core_inputs(*args, core=c) for c in range(N_CORES)]

    trace = bool(int(os.environ.get("BASS_KERNEL_TRACE", "0")))
    res = run_bass_kernel_spmd(
        nc,
        in_maps,
        core_ids=list(range(N_CORES)),
        trace=trace,
    )
    if trace:
        kernel.last_results = res

    out = np.empty((B, SQ, H * D_V), dtype=np.float32)
    for c in range(N_CORES):
        b = c // 2
        hh = c % 2
        out[b, :, HALF * hh : HALF * (hh + 1)] = res.results[c]["out"]
    return out


# revision 16
# speedup vs baseline: 1.0388x; 1.0173x over previous
"""Trainium2 Bass kernel for nn_AttentionSeqToMasked (dense transformer attention).

Full-input contract: kernel(**inputs) takes the unsharded numpy inputs and
returns the full [B, SQ, H*D_V] float32 output.

Sharding (8 cores): data parallel over batch (B=4 -> 2 cores per batch) x
tensor parallel over heads (16 heads -> 8 per core). Each core computes
attention for one (batch, head-half) pair; host gathers the slices.

Per-core dataflow (all matmuls bf16 inputs, fp32 PSUM accumulation):
  - Host pre-transposes activations to X^T [D_PRE, S] bf16 so the contraction
    dim (D_PRE) lands on SBUF partitions with fully-contiguous DMA loads.
  - Projections compute q^T/k^T = W^T @ X^T directly (head-dim on partitions),
    v in natural [s, d_v] layout with a ones-column appended via the weight
    matrix (zero weight column + bias 1.0).
  - Scores are computed transposed: scoresT[k, q] = kT.T @ qT, two heads
    packed into the 128x128 PE array per matmul pair (d_head=64 row groups).
  - Key-mask folds into the exp as a per-partition bias (0 or -30000);
    1/sqrt(d) folds into the exp scale. No max-subtraction is needed
    (logits are O(1) by construction; exp cannot overflow fp32).
  - AV matmul contracts exp(scores)T with [v | ones]: row 64 of the psum is
    the softmax denominator, computed for free alongside the numerator.
  - A final PE transpose returns [q, d_v+1] tiles; VectorE multiplies by the
    reciprocal denominator and the result DMAs straight to DRAM.

Scheduling: projection work for pair p+1 is chopped into ~1.7us psum-chunks
and interleaved into pair p's attention loop every 4 key-tiles, keeping the
TensorE fed while ScalarE (exp) is the steady-state bottleneck.
"""

import os
from contextlib import ExitStack

import numpy as np
import ml_dtypes

import concourse.bass as bass
import concourse.bacc as bacc
import concourse.mybir as mybir
import concourse.tile as tile
from concourse.bass_utils import run_bass_kernel_spmd
from concourse.masks import make_identity

# Problem shape (hardcoded per contract)
B, SQ, SK = 4, 2048, 2048
D_PRE = 1024
H, D_QK, D_V = 16, 64, 64
N_CORES = 8
HALF = (H // 2) * D_QK  # 512 columns of the projection handled per core
N_PAIRS = 4  # head pairs per core
S_CHUNK = 512  # moving free-dim per matmul
N_DT = D_PRE // 128  # d_pre tiles of 128
N_KT = SK // 128  # key tiles of 128
N_QC = SQ // S_CHUNK  # query chunks of 512
MASK_NEG = -30000.0

F32 = mybir.dt.float32
BF16 = mybir.dt.bfloat16
BF16_NP = np.dtype(ml_dtypes.bfloat16)

_COMPILED = None


def _build_program():
    nc = bacc.Bacc("TRN2", target_bir_lowering=False, debug=False)

    # DRAM I/O (names are the in_map keys)
    xq = nc.dram_tensor("xq", [D_PRE, SQ], BF16, kind="ExternalInput").ap()
    xk = nc.dram_tensor("xk", [D_PRE, SK], BF16, kind="ExternalInput").ap()
    xv = nc.dram_tensor("xv", [D_PRE, SK], BF16, kind="ExternalInput").ap()
    wq = nc.dram_tensor("wq", [D_PRE, HALF], BF16, kind="ExternalInput").ap()
    wk = nc.dram_tensor("wk", [D_PRE, HALF], BF16, kind="ExternalInput").ap()
    # v weights with a zero column appended per head (ones column generator)
    wv = nc.dram_tensor("wv", [D_PRE, N_PAIRS * 130], BF16, kind="ExternalInput").ap()
    bq = nc.dram_tensor("bq", [128, N_PAIRS], F32, kind="ExternalInput").ap()
    bk = nc.dram_tensor("bk", [128, N_PAIRS], F32, kind="ExternalInput").ap()
    bv = nc.dram_tensor("bv", [128, N_PAIRS * 130], F32, kind="ExternalInput").ap()
    mb = nc.dram_tensor("mb", [128, N_KT], F32, kind="ExternalInput").ap()
    out = nc.dram_tensor("out", [SQ, HALF], F32, kind="ExternalOutput").ap()

    with tile.TileContext(nc) as tc:
        _emit(tc, xq, xk, xv, wq, wk, wv, bq, bk, bv, mb, out)

    nc.compile()
    return nc


def _emit(tc, xq, xk, xv, wq, wk, wv, bq, bk, bv, mb, out):
    nc = tc.nc

    with ExitStack() as ctx:
        # ---- pools ----
        xp = ctx.enter_context(tc.tile_pool(name="x", bufs=3 * N_DT))
        wp = ctx.enter_context(tc.tile_pool(name="w", bufs=1))
        cp = ctx.enter_context(tc.tile_pool(name="const", bufs=1))
        qkvp = ctx.enter_context(tc.tile_pool(name="qkv", bufs=1))
        expp = ctx.enter_context(tc.tile_pool(name="exp", bufs=3))
        avtp = ctx.enter_context(tc.tile_pool(name="avt", bufs=2))
        stgp = ctx.enter_context(tc.tile_pool(name="stg", bufs=8))
        rp = ctx.enter_context(tc.tile_pool(name="recip", bufs=8))

        proj_ps = ctx.enter_context(tc.tile_pool(name="proj_ps", bufs=1, space="PSUM"))
        sc_ps = ctx.enter_context(tc.tile_pool(name="sc_ps", bufs=2, space="PSUM"))
        av_ps = ctx.enter_context(tc.tile_pool(name="av_ps", bufs=2, space="PSUM"))
        tp_ps = ctx.enter_context(tc.tile_pool(name="tp_ps", bufs=1, space="PSUM"))

        # ---- constants ----
        ident = cp.tile([128, 128], F32, name="ident")
        make_identity(nc, ident)
        mb_sb = cp.tile([128, N_KT], F32, name="mb_sb")
        nc.sync.dma_start(mb_sb, mb)
        bq_sb = cp.tile([128, N_PAIRS], F32, name="bq_sb")
        nc.sync.dma_start(bq_sb, bq)
        bk_sb = cp.tile([128, N_PAIRS], F32, name="bk_sb")
        nc.sync.dma_start(bk_sb, bk)
        bv_sb = cp.tile([128, N_PAIRS * 130], F32, name="bv_sb")
        nc.sync.dma_start(bv_sb, bv)

        # ---- streamed loads, ordered so the q projection can start after
        # only wq+xq have landed ----
        def load_x(xap, pfx):
            ts = []
            for dt_i in range(N_DT):
                t = xp.tile([128, SQ], BF16, name=f"{pfx}{dt_i}", tag="x")
                nc.sync.dma_start(t, xap[dt_i * 128 : (dt_i + 1) * 128, :])
                ts.append(t)
            return ts

        def load_w(wap, pfx, width):
            ts = []
            for dt_i in range(N_DT):
                t = wp.tile([128, width], BF16, name=f"{pfx}{dt_i}", tag=f"{pfx}{dt_i}")
                nc.sync.dma_start(t, wap[dt_i * 128 : (dt_i + 1) * 128, :])
                ts.append(t)
            return ts

        wq_sb = load_w(wq, "wq", HALF)
        xq_sb = load_x(xq, "xq")
        wk_sb = load_w(wk, "wk", HALF)
        xk_sb = load_x(xk, "xk")
        wv_sb = load_w(wv, "wv", N_PAIRS * 130)
        xv_sb = load_x(xv, "xv")

        v_tiles = {}  # (pair, kt) -> [128, 130] bf16 tile
        qkT = {}  # (pfx, pair) -> [128, SQ] bf16 tile

        def qk_tile(pfx, pair):
            if (pfx, pair) not in qkT:
                qkT[(pfx, pair)] = qkvp.tile(
                    [128, SQ], BF16, name=f"{pfx}T{pair}", tag=f"{pfx}T", bufs=2
                )
            return qkT[(pfx, pair)]

        proj_ps_open = {}

        def emit_qk_chunk(pair, pfx, qc, half=None):
            # one [128, 512] projection chunk: 8 accumulating MMs + bias copy.
            # half=0/1 emits only the first/second 4 contraction MMs (filler
            # granularity); half=None emits the whole chunk.
            dst = qk_tile(pfx, pair)
            w_sb = wq_sb if pfx == "q" else wk_sb
            b_sb = bq_sb if pfx == "q" else bk_sb
            x_sb = xq_sb if pfx == "q" else xk_sb
            key = (pair, pfx, qc)
            if half == 1:
                ps = proj_ps_open.pop(key)
            else:
                ps = proj_ps.tile(
                    [128, S_CHUNK], F32, name=f"{pfx}ps{pair}_{qc}", tag="proj"
                )
            dts = range(N_DT) if half is None else range(half * 4, half * 4 + 4)
            for dt_i in dts:
                nc.tensor.matmul(
                    ps,
                    lhsT=w_sb[dt_i][:, pair * 128 : (pair + 1) * 128],
                    rhs=x_sb[dt_i][:, qc * S_CHUNK : (qc + 1) * S_CHUNK],
                    start=(dt_i == 0),
                    stop=(dt_i == N_DT - 1),
                )
            if half == 0:
                proj_ps_open[key] = ps
            else:
                nc.vector.tensor_scalar_add(
                    dst[:, qc * S_CHUNK : (qc + 1) * S_CHUNK],
                    ps,
                    b_sb[:, pair : pair + 1],
                )

        def emit_v_chunk(g, st):
            # v projection for pairs (2g, 2g+1), one key tile: N=260 matmuls
            ps = proj_ps.tile([128, S_CHUNK], F32, name=f"vps{g}_{st}", tag="proj")
            for dt_i in range(N_DT):
                nc.tensor.matmul(
                    ps[:, 0:260],
                    lhsT=xv_sb[dt_i][:, st * 128 : (st + 1) * 128],
                    rhs=wv_sb[dt_i][:, g * 260 : (g + 1) * 260],
                    start=(dt_i == 0),
                    stop=(dt_i == N_DT - 1),
                )
            for j in range(2):
                pair = 2 * g + j
                vt = qkvp.tile(
                    [128, 130], BF16, name=f"v{pair}_{st}", tag="v", bufs=4 * N_KT
                )
                nc.vector.tensor_add(
                    vt,
                    ps[:, j * 130 : (j + 1) * 130],
                    bv_sb[:, pair * 130 : (pair + 1) * 130],
                )
                v_tiles[(pair, st)] = vt

        # filler queue: projection chunk units (~0.85us of PE each), popped as
        # TensorE filler inside the attention stream. qk chunks are split into
        # 4-MM halves to match the per-slot budget. Ordering constraints:
        # qk(p) before pair-p attention, vg1 before pair-2 attention.
        filler = []

        def _qk_half(pair, pfx, qc, half):
            return lambda: emit_qk_chunk(pair, pfx, qc, half)

        for pfx in ("q", "k"):
            for c in range(N_QC):
                filler.append(_qk_half(1, pfx, c, 0))
                filler.append(_qk_half(1, pfx, c, 1))
        filler += [(lambda st=st: emit_v_chunk(1, st)) for st in range(N_KT)]
        for pfx in ("q", "k"):
            for c in range(N_QC):
                filler.append(_qk_half(2, pfx, c, 0))
                filler.append(_qk_half(2, pfx, c, 1))
        for pfx in ("q", "k"):
            for c in range(N_QC):
                filler.append(_qk_half(3, pfx, c, 0))
                filler.append(_qk_half(3, pfx, c, 1))

        def pop_filler():
            if filler:
                filler.pop(0)()

        # prologue: pair-0 projections, ordered so the first scores tile
        # (needing only the qc=0 chunks of qT0/kT0) unblocks ASAP
        emit_qk_chunk(0, "q", 0)
        emit_qk_chunk(0, "k", 0)
        for c in range(1, N_QC):
            emit_qk_chunk(0, "q", c)
        for c in range(1, N_QC):
            emit_qk_chunk(0, "k", c)
        for st in range(N_KT):
            emit_v_chunk(0, st)

        # ---- software-pipelined attention stream over (pair, qc, kt) ----
        iters = [
            (pair, qc, kt)
            for pair in range(N_PAIRS)
            for qc in range(N_QC)
            for kt in range(N_KT)
        ]
        sc_map = {}
        av_map = {}

        def emit_scores(i):
            pair, qc, kt = iters[i]
            qT = qk_tile("q", pair)
            kT = qk_tile("k", pair)
            sc = sc_ps.tile([128, 1024], F32, name=f"sc{pair}_{qc}_{kt}", tag="sc")
            # scoresT for heads A and B, packed in PE row groups
            nc.tensor.matmul(
                sc[:, 0:512],
                lhsT=kT[0:64, kt * 128 : (kt + 1) * 128],
                rhs=qT[0:64, qc * S_CHUNK : (qc + 1) * S_CHUNK],
                start=True,
                stop=True,
            )
            nc.tensor.matmul(
                sc[:, 512:1024],
                lhsT=kT[64:128, kt * 128 : (kt + 1) * 128],
                rhs=qT[64:128, qc * S_CHUNK : (qc + 1) * S_CHUNK],
                start=True,
                stop=True,
            )
            sc_map[i] = sc

        def emit_epilogue(pair, qc, av_a, av_b):
            # transpose back to [q, d_v], normalize, store
            stgs = [
                stgp.tile([128, 128], F32, name=f"st{pair}_{qc}_{u}", tag="stg")
                for u in range(4)
            ]
            for h_i, av in enumerate((av_a, av_b)):
                avt = avtp.tile(
                    [65, S_CHUNK], F32, name=f"avt{pair}_{qc}_{h_i}", tag="avt"
                )
                nc.vector.tensor_copy(avt, av)
                tp = tp_ps.tile([128, 260], F32, name=f"tp{pair}_{qc}_{h_i}", tag="tp")
                for u in range(4):
                    nc.tensor.transpose(
                        tp[:, u * 65 : u * 65 + 65],
                        avt[:, u * 128 : (u + 1) * 128],
                        ident[0:65, 0:65],
                    )
                for u in range(4):
                    rc = rp.tile([128, 1], F32, name=f"rc{pair}_{qc}_{h_i}_{u}", tag="rc")
                    nc.vector.reciprocal(rc, tp[:, u * 65 + 64 : u * 65 + 65])
                    nc.vector.tensor_scalar_mul(
                        stgs[u][:, h_i * 64 : (h_i + 1) * 64],
                        tp[:, u * 65 : u * 65 + 64],
                        rc,
                    )
            for u in range(4):
                qt = qc * 4 + u
                nc.sync.dma_start(
                    out[qt * 128 : (qt + 1) * 128, pair * 128 : (pair + 1) * 128],
                    stgs[u],
                )

        def emit_av(pair, qc, kt, ex):
            if kt == 0:
                av_map[(pair, qc)] = (
                    av_ps.tile([65, S_CHUNK], F32, name=f"ava{pair}_{qc}", tag="av"),
                    av_ps.tile([65, S_CHUNK], F32, name=f"avb{pair}_{qc}", tag="av"),
                )
            av_a, av_b = av_map[(pair, qc)]
            nc.tensor.matmul(
                av_a,
                lhsT=v_tiles[(pair, kt)][:, 0:65],
                rhs=ex[:, 0:512],
                start=(kt == 0),
                stop=(kt == N_KT - 1),
            )
            nc.tensor.matmul(
                av_b,
                lhsT=v_tiles[(pair, kt)][:, 65:130],
                rhs=ex[:, 512:1024],
                start=(kt == 0),
                stop=(kt == N_KT - 1),
            )

        emit_scores(0)
        emit_scores(1)
        deferred_av = []  # (kt, ex) for the (0, 0) warmup chunk
        for i, (pair, qc, kt) in enumerate(iters):
            ex = expp.tile(
                [128, 1024], BF16, name=f"ex{pair}_{qc}_{kt}", tag="ex", bufs=12
            )
            nc.scalar.activation(
                ex,
                sc_map.pop(i),
                mybir.ActivationFunctionType.Exp,
                bias=mb_sb[:, kt : kt + 1],
                scale=0.125,
            )
            if i < N_KT:
                # warmup (pair 0, qc 0): run all exps ahead of the AVs so the
                # ScalarE pipeline starts before the v projection has landed
                # (v is gated on the full xv DMA, ~50us in)
                deferred_av.append((kt, ex))
                if i + 2 < len(iters):
                    emit_scores(i + 2)
                if kt % 2 == 1 and kt <= 11:
                    pop_filler()
                if kt == N_KT - 1:
                    for dkt, dex in deferred_av:
                        emit_av(0, 0, dkt, dex)
                    deferred_av.clear()
                    emit_epilogue(0, 0, *av_map.pop((0, 0)))
                continue
            emit_av(pair, qc, kt, ex)
            if i + 2 < len(iters):
                emit_scores(i + 2)
            if pair <= 1 and kt % 2 == 1 and kt <= 11:
                pop_filler()
            elif pair == 2 and kt % 4 == 1:
                pop_filler()
            if kt == N_KT - 1:
                emit_epilogue(pair, qc, *av_map.pop((pair, qc)))

        assert not filler, f"{len(filler)} filler chunks left unscheduled"


def _prep_core_inputs(pre_qs, pre_ks, pre_vs, k_mask, q_w, q_b, k_w, k_b, v_w, v_b, core):
    b = core // 2
    hh = core % 2
    cols = slice(HALF * hh, HALF * (hh + 1))

    xq = np.ascontiguousarray(pre_qs[b].T).astype(BF16_NP)
    xk = np.ascontiguousarray(pre_ks[b].T).astype(BF16_NP)
    xv = np.ascontiguousarray(pre_vs[b].T).astype(BF16_NP)
    wq = np.ascontiguousarray(q_w[:, cols]).astype(BF16_NP)
    wk = np.ascontiguousarray(k_w[:, cols]).astype(BF16_NP)

    wv_core = v_w[:, cols].astype(np.float32)
    wv = np.zeros((D_PRE, N_PAIRS * 130), dtype=np.float32)
    bv_core = v_b[cols].astype(np.float32)
    bv_ext = np.zeros(N_PAIRS * 130, dtype=np.float32)
    for p in range(N_PAIRS):
        wv[:, p * 130 : p * 130 + 64] = wv_core[:, p * 128 : p * 128 + 64]
        wv[:, p * 130 + 65 : p * 130 + 129] = wv_core[:, p * 128 + 64 : p * 128 + 128]
        bv_ext[p * 130 : p * 130 + 64] = bv_core[p * 128 : p * 128 + 64]
        bv_ext[p * 130 + 64] = 1.0
        bv_ext[p * 130 + 65 : p * 130 + 129] = bv_core[p * 128 + 64 : p * 128 + 128]
        bv_ext[p * 130 + 129] = 1.0

    bq = np.ascontiguousarray(q_b[cols].astype(np.float32).reshape(N_PAIRS, 128).T)
    bk = np.ascontiguousarray(k_b[cols].astype(np.float32).reshape(N_PAIRS, 128).T)
    bv_full = np.ascontiguousarray(np.tile(bv_ext[None, :], (128, 1)))

    # mask True -> 0.0, False -> MASK_NEG
    mbias = np.where(k_mask[b], 0.0, MASK_NEG).astype(np.float32)
    mb = np.ascontiguousarray(mbias.reshape(N_KT, 128).T)

    return {
        "xq": xq,
        "xk": xk,
        "xv": xv,
        "wq": wq,
        "wk": wk,
        "wv": wv.astype(BF16_NP),
        "bq": bq,
        "bk": bk,
        "bv": bv_full,
        "mb": mb,
    }


def kernel(pre_qs, pre_ks, pre_vs, k_mask, q_w, q_b, k_w, k_b, v_w, v_b):
    global _COMPILED
    args = (pre_qs, pre_ks, pre_vs, k_mask, q_w, q_b, k_w, k_b, v_w, v_b)
    args = tuple(np.asarray(a) for a in args)

    if _COMPILED is None:
        _COMPILED = _build_program()
    nc = _COMPILED

    in_maps = [_prep_core_inputs(*args, core=c) for c in range(N_CORES)]

    trace = bool(int(os.environ.get("BASS_KERNEL_TRACE", "0")))
    res = run_bass_kernel_spmd(
        nc,
        in_maps,
        core_ids=list(range(N_CORES)),
        trace=trace,
    )
    if trace:
        kernel.last_results = res

    out = np.empty((B, SQ, H * D_V), dtype=np.float32)
    for c in range(N_CORES):
        b = c // 2
        hh = c % 2
        out[b, :, HALF * hh : HALF * (hh + 1)] = res.results[c]["out"]
    return out
